# revision 6
# baseline (speedup 1.0000x reference)
"""Trainium2 Bass kernel for nn_Bottleneck (sparse-conv bottleneck / GNN message passing).

Data-parallel over points (8 cores x 12500 points):
  conv1: h = relu(LN(feats @ W1)) per-core shard
  AllGather h shards -> full h table [100000, 64] in each core's DRAM
  conv2: gather h[neighbor_idx] (27 rows/point) via indirect DMA,
         PE-transpose to channel-major, contract (k,c)=1728 in 14 chunks
  conv3: h2 @ W3 -> LN -> +feats residual -> relu

LayerNorm gamma/beta are ones/zeros in this problem spec -> skipped.
"""
import numpy as np

N = 100000
C_IN = 256
C_MID = 64
C_OUT = 256
K = 27
EPS = 1e-6
NCORES = 8
NT = N // NCORES          # 12500 points per core
P = 128
NTILES = (NT + P - 1) // P  # 98 (last tile 84 rows)
KC = K * C_MID              # 1728
NCHUNK = (KC + P - 1) // P  # 14 (last chunk 64 wide)

_RUNNER = {}


def _build(debug_no_gather=False, debug_no_collective=False, debug_ntiles=None):
    import concourse.bass as bass
    import concourse.tile as tile
    from concourse import bacc, mybir
    from concourse.masks import make_identity
    ntiles = NTILES if debug_ntiles is None else debug_ntiles

    f32 = mybir.dt.float32
    i32 = mybir.dt.int32

    nc = bacc.Bacc(None, target_bir_lowering=False, num_devices=NCORES,
                   dynamic_dma_scratch_size=65536)

    featsT = nc.dram_tensor("featsT", [C_IN, NT], f32, kind="ExternalInput")
    feats = nc.dram_tensor("feats", [NT, C_IN], f32, kind="ExternalInput")
    nbr = nc.dram_tensor("nbr", [NT, K], i32, kind="ExternalInput")
    W1 = nc.dram_tensor("W1", [C_IN, C_MID], f32, kind="ExternalInput")
    W2f = nc.dram_tensor("W2f", [KC, C_MID], f32, kind="ExternalInput")
    W3 = nc.dram_tensor("W3", [C_MID, C_OUT], f32, kind="ExternalInput")
    out = nc.dram_tensor("out", [NT, C_OUT], f32, kind="ExternalOutput")

    with tile.TileContext(nc) as tc:
        with (
            tc.tile_pool(name="dram", bufs=1, space="DRAM") as dram,
            tc.tile_pool(name="consts", bufs=1) as consts,
            tc.tile_pool(name="io1", bufs=3) as io1,
            tc.tile_pool(name="ln", bufs=4) as lnp,
            tc.tile_pool(name="gp", bufs=2) as gp,
            tc.tile_pool(name="gt", bufs=3) as gtp,
            tc.tile_pool(name="io3", bufs=3) as io3,
            tc.tile_pool(name="ps1", bufs=2, space="PSUM") as ps1,
            tc.tile_pool(name="pst", bufs=2, space="PSUM") as pst,
            tc.tile_pool(name="ps2", bufs=2, space="PSUM") as ps2,
            tc.tile_pool(name="ps3", bufs=2, space="PSUM") as ps3,
        ):
            h_shard = dram.tile([NT, C_MID], f32)
            h_full = dram.tile([N, C_MID], f32, addr_space="Shared")

            # constants
            W1s = consts.tile([P, 2, C_MID], f32)
            nc.sync.dma_start(out=W1s[:, 0, :], in_=W1[0:P, :])
            nc.sync.dma_start(out=W1s[:, 1, :], in_=W1[P:2*P, :])
            W2s = consts.tile([P, NCHUNK, C_MID], f32)
            for j in range(NCHUNK):
                w = min(P, KC - j * P)
                nc.sync.dma_start(out=W2s[:w, j, :], in_=W2f[j*P:j*P+w, :])
            W3s = consts.tile([C_MID, C_OUT], f32)
            nc.sync.dma_start(out=W3s[:, :], in_=W3[:, :])
            ident = consts.tile([P, P], f32)
            make_identity(nc, ident[:])
            epst = consts.tile([P, 1], f32)
            nc.vector.memset(epst[:], EPS)

            def layernorm(x_ap, o_ap, T, relu):
                """o = LN(x) over the free dim (gamma=1, beta=0), optional relu."""
                stats = lnp.tile([P, 6], f32, tag="stats")
                mv = lnp.tile([P, 2], f32, tag="mv")
                nc.vector.bn_stats(out=stats[:T, :], in_=x_ap)
                nc.vector.bn_aggr(out=mv[:T, :], in_=stats[:T, :])
                rstd = lnp.tile([P, 1], f32, tag="rstd")
                nc.scalar.activation(
                    out=rstd[:T, :], in_=mv[:T, 1:2],
                    func=mybir.ActivationFunctionType.Sqrt,
                    bias=epst[:T], scale=1.0, alpha=0.0)
                nc.vector.reciprocal(out=rstd[:T, :], in_=rstd[:T, :])
                nc.vector.tensor_scalar(
                    out=o_ap, in0=x_ap,
                    scalar1=mv[:T, 0:1], scalar2=rstd[:T, :],
                    op0=mybir.AluOpType.subtract, op1=mybir.AluOpType.mult)
                if relu:
                    nc.scalar.activation(
                        out=o_ap, in_=o_ap,
                        func=mybir.ActivationFunctionType.Relu)

            # ---------------- phase 1: conv1 ----------------
            for t in range(ntiles):
                r0 = t * P
                T = min(P, NT - r0)
                fT = io1.tile([P, 2, P], f32, tag="fT")
                nc.sync.dma_start(out=fT[:, 0, :T], in_=featsT[0:P, r0:r0+T])
                nc.sync.dma_start(out=fT[:, 1, :T], in_=featsT[P:2*P, r0:r0+T])
                psum1 = ps1.tile([P, C_MID], f32, tag="psum1")
                for j in range(2):
                    nc.tensor.matmul(
                        out=psum1[:T, :], lhsT=fT[:, j, :T], rhs=W1s[:, j, :],
                        start=(j == 0), stop=(j == 1))
                h_t = io1.tile([P, C_MID], f32, tag="h_t")
                layernorm(psum1[:T, :], h_t[:T, :], T, relu=True)
                nc.sync.dma_start(out=h_shard[r0:r0+T, :], in_=h_t[:T, :])

            # ---------------- phase 2: allgather ----------------
            if debug_no_collective:
                for c in range(NCORES):
                    nc.sync.dma_start(out=h_full[c*NT:(c+1)*NT, :][0:NT, :],
                                      in_=h_shard[:, :]) if c == 0 else None
            else:
                nc.gpsimd.collective_compute(
                    "AllGather", mybir.AluOpType.bypass,
                    replica_groups=[list(range(NCORES))],
                    ins=[h_shard[:, :].opt()],
                    outs=[h_full[:, :].opt()],
                )

            # ---------------- phase 3: conv2 + conv3 ----------------
            for t in range(ntiles):
                r0 = t * P
                T = min(P, NT - r0)
                idx_t = io3.tile([P, K], i32, tag="idx")
                nc.sync.dma_start(out=idx_t[:T, :], in_=nbr[r0:r0+T, :])
                G = gp.tile([P, K, C_MID], f32, tag="G")
                if debug_no_gather:
                    nc.sync.dma_start(
                        out=G[:T].rearrange("p k d -> p (k d)")[:, 0:C_MID],
                        in_=h_full[r0:r0+T, :])
                    nc.vector.memset(G[:T, 1:K, :], 0.01)
                else:
                    for k in range(K):
                        nc.gpsimd.indirect_dma_start(
                            out=G[:T, k, :], out_offset=None,
                            in_=h_full[:, :],
                            in_offset=bass.IndirectOffsetOnAxis(
                                ap=idx_t[:T, k:k+1], axis=0))
                Gf = G[:T].rearrange("p k d -> p (k d)")
                psum2 = ps2.tile([P, C_MID], f32, tag="psum2")
                for j in range(NCHUNK):
                    w = min(P, KC - j * P)
                    ps_t = pst.tile([P, P], f32, tag="ps_t")
                    nc.tensor.transpose(
                        out=ps_t[:w, :T], in_=Gf[:, j*P:j*P+w],
                        identity=ident[:T, :T])
                    gt = gtp.tile([P, P], f32, tag="gt")
                    nc.vector.tensor_copy(out=gt[:w, :T], in_=ps_t[:w, :T])
                    nc.tensor.matmul(
                        out=psum2[:T, :], lhsT=gt[:w, :T], rhs=W2s[:w, j, :],
                        start=(j == 0), stop=(j == NCHUNK - 1))
                h2 = io3.tile([P, C_MID], f32, tag="h2")
                layernorm(psum2[:T, :], h2[:T, :], T, relu=True)
                ps_h2t = pst.tile([P, P], f32, tag="ps_t")
                nc.tensor.transpose(
                    out=ps_h2t[:C_MID, :T], in_=h2[:T, :],
                    identity=ident[:T, :T])
                h2t = io3.tile([C_MID, P], f32, tag="h2t")
                nc.vector.tensor_copy(out=h2t[:, :T], in_=ps_h2t[:C_MID, :T])
                psum3 = ps3.tile([P, C_OUT], f32, tag="psum3")
                nc.tensor.matmul(
                    out=psum3[:T, :], lhsT=h2t[:, :T], rhs=W3s[:, :],
                    start=True, stop=True)
                o_t = io3.tile([P, C_OUT], f32, tag="o_t")
                layernorm(psum3[:T, :], o_t[:T, :], T, relu=False)
                f_t = io3.tile([P, C_IN], f32, tag="f_t")
                nc.sync.dma_start(out=f_t[:T, :], in_=feats[r0:r0+T, :])
                nc.vector.tensor_add(out=o_t[:T, :], in0=o_t[:T, :], in1=f_t[:T, :])
                nc.scalar.activation(
                    out=o_t[:T, :], in_=o_t[:T, :],
                    func=mybir.ActivationFunctionType.Relu)
                nc.sync.dma_start(out=out[r0:r0+T, :], in_=o_t[:T, :])

    nc.compile()
    return nc


def _make_runner(nc, n_cores):
    import jax
    from jax.sharding import Mesh, PartitionSpec
    from jax.experimental.shard_map import shard_map
    import concourse.mybir as mybir
    from concourse.bass2jax import (
        _bass_exec_p, install_neuronx_cc_hook, partition_id_tensor)

    install_neuronx_cc_hook()
    partition_name = nc.partition_id_tensor.name if nc.partition_id_tensor else None

    in_names, out_names, out_avals, zero_outs = [], [], [], []
    for alloc in nc.m.functions[0].allocations:
        if not isinstance(alloc, mybir.MemoryLocationSet):
            continue
        name = alloc.memorylocations[0].name
        if alloc.kind == "ExternalInput":
            if name != partition_name:
                in_names.append(name)
        elif alloc.kind == "ExternalOutput":
            shape = tuple(alloc.tensor_shape)
            dtype = mybir.dt.np(alloc.dtype)
            out_avals.append(jax.core.ShapedArray(shape, dtype))
            out_names.append(name)
            zero_outs.append(np.zeros(shape, dtype))
    n_params = len(in_names)
    n_outs = len(out_avals)
    all_in_names = list(in_names) + list(out_names)
    if partition_name is not None:
        all_in_names.append(partition_name)
    donate = tuple(range(n_params, n_params + n_outs))

    def _body(*args):
        operands = list(args)
        if partition_name is not None:
            operands.append(partition_id_tensor())
        outs = _bass_exec_p.bind(
            *operands,
            out_avals=tuple(out_avals),
            in_names=tuple(all_in_names),
            out_names=tuple(out_names),
            lowering_input_output_aliases=(),
            sim_require_finite=True,
            sim_require_nnan=True,
            nc=nc,
        )
        return tuple(outs)

    devices = jax.devices()[:n_cores]
    mesh = Mesh(np.asarray(devices), ("core",))
    in_specs = (PartitionSpec("core"),) * (n_params + n_outs)
    out_specs = (PartitionSpec("core"),) * n_outs
    sharded = jax.jit(
        shard_map(_body, mesh=mesh, in_specs=in_specs, out_specs=out_specs,
                  check_rep=False),
        donate_argnums=donate, keep_unused=True,
    )

    def fn(in_maps):
        per_core = [[np.asarray(m[name]) for name in in_names] for m in in_maps]
        concat_in = [np.concatenate([per_core[c][i] for c in range(n_cores)], axis=0)
                     for i in range(n_params)]
        concat_zeros = [np.zeros((n_cores * z.shape[0], *z.shape[1:]), z.dtype)
                        for z in zero_outs]
        out_arrs = sharded(*concat_in, *concat_zeros)
        out_arrs = [np.asarray(a) for a in out_arrs]
        return [
            {name: out_arrs[i].reshape(n_cores, *out_avals[i].shape)[c]
             for i, name in enumerate(out_names)}
            for c in range(n_cores)
        ]

    return fn


def _get_runner():
    if "fn" not in _RUNNER:
        nc = _build()
        _RUNNER["fn"] = _make_runner(nc, NCORES)
    return _RUNNER["fn"]


def kernel(feats, neighbor_idx, W1, g1, b1, W2, g2, b2, W3, g3, b3):
    feats = np.asarray(feats, dtype=np.float32)
    neighbor_idx = np.asarray(neighbor_idx, dtype=np.int32)
    W1 = np.asarray(W1, dtype=np.float32)
    W2 = np.asarray(W2, dtype=np.float32)
    W3 = np.asarray(W3, dtype=np.float32)
    W2f = np.ascontiguousarray(W2.reshape(KC, C_MID))
    featsT = np.ascontiguousarray(feats.T)

    fn = _get_runner()
    in_maps = []
    for c in range(NCORES):
        sl = slice(c * NT, (c + 1) * NT)
        in_maps.append({
            "featsT": np.ascontiguousarray(featsT[:, sl]),
            "feats": feats[sl],
            "nbr": neighbor_idx[sl],
            "W1": W1, "W2f": W2f, "W3": W3,
        })
    res = fn(in_maps)
    return np.concatenate([res[c]["out"] for c in range(NCORES)], axis=0)


# revision 16
# speedup vs baseline: 1.2140x; 1.2140x over previous
"""Trainium2 Bass kernel for nn_Bottleneck (sparse-conv bottleneck / GNN message passing).

Data-parallel over points (8 cores x 12500 points):
  conv1: h = relu(LN(feats @ W1)) per-core shard
  AllGather h shards -> full h table [100000, 64] in each core's DRAM
  conv2: gather h[neighbor_idx] (27 rows/point) via indirect DMA,
         PE-transpose to channel-major, contract (k,c)=1728 in 14 chunks
  conv3: h2 @ W3 -> LN -> +feats residual -> relu

LayerNorm gamma/beta are ones/zeros in this problem spec -> skipped.
"""
import numpy as np

N = 100000
C_IN = 256
C_MID = 64
C_OUT = 256
K = 27
EPS = 1e-6
NCORES = 8
NT = N // NCORES          # 12500 points per core
P = 128
NTILES = (NT + P - 1) // P  # 98 (last tile 84 rows)
KC = K * C_MID              # 1728
NCHUNK = (KC + P - 1) // P  # 14 (last chunk 64 wide)

SLOTS = K * P      # 3456 gather slots per tile (slot i = k*128 + token)
SCHUNK = 34        # staging capacity in 128-row chunks (4352 slots)
NSEG = 4
SEGW = 25000       # value-segment width (< 32768 for int16 local indices)

_RUNNER = {}


def _pack16(flat):
    """flat [16*cols] int16 -> wrapped [128, cols] (16-partition wrap, 8x replicated)."""
    cols = len(flat) // 16
    w = flat.reshape(cols, 16).T.astype(np.int16)
    return np.ascontiguousarray(np.tile(w, (8, 1)))


def _prep_gather(nbr_all):
    """Build per-tile sorted/segmented gather#1 index streams, unsort gather#2
    streams, and uniform (cross-core) call metadata."""
    counts = np.zeros((NCORES, NTILES, NSEG), np.int64)
    percore = []
    for c in range(NCORES):
        shard = nbr_all[c*NT:(c+1)*NT]
        pad = np.zeros((NTILES*P, K), np.int32)
        pad[:NT] = shard
        tiles = pad.reshape(NTILES, P, K).transpose(0, 2, 1).reshape(NTILES, SLOTS)
        tl = []
        for t in range(NTILES):
            vals = tiles[t]
            order = np.argsort(vals, kind="stable")
            sv = vals[order]
            b = np.searchsorted(sv, [SEGW, 2*SEGW, 3*SEGW]).astype(np.int64)
            bounds = np.array([0, b[0], b[1], b[2], SLOTS], np.int64)
            counts[c, t] = np.diff(bounds)
            tl.append((sv, order, bounds))
        percore.append(tl)
    pcnt = ((counts.max(axis=0) + P - 1) // P) * P        # [NTILES, NSEG]
    assert (pcnt.sum(axis=1) <= SCHUNK * P).all()

    meta_tiles = []
    scol = 0
    for t in range(NTILES):
        calls = []
        soff = 0
        coff = 0
        segoff = {}
        for q in range(NSEG):
            c_ = int(pcnt[t, q])
            if c_ == 0:
                continue
            calls.append((c_, q*SEGW, coff, soff))
            segoff[q] = (coff, soff)
            soff += c_ // 16
            coff += c_ // P
        meta_tiles.append({"calls": calls, "scol0": scol, "tcols": soff,
                           "segoff": segoff})
        scol += soff
    meta = {"tiles": meta_tiles, "siw": scol,
            "max_tcols": max(mt["tcols"] for mt in meta_tiles)}

    sidxs, uidxs = [], []
    for c in range(NCORES):
        sflat = np.full((meta["siw"]*16,), -1, np.int16)
        uflat = np.zeros((NTILES*SLOTS,), np.int16)
        for t in range(NTILES):
            sv, order, bounds = percore[c][t]
            mt = meta_tiles[t]
            inv = np.empty(SLOTS, np.int64)
            inv[order] = np.arange(SLOTS)
            sarr = inv                                   # sorted rank per slot
            q = ((sarr >= bounds[1]).astype(np.int64)
                 + (sarr >= bounds[2]) + (sarr >= bounds[3]))
            j = sarr - bounds[q]
            coffq = np.array([mt["segoff"].get(qq, (0, 0))[0]
                              for qq in range(NSEG)], np.int64)[q]
            uflat[t*SLOTS:(t+1)*SLOTS] = ((j % P) * SCHUNK + coffq + j // P
                                          ).astype(np.int16)
            base16 = mt["scol0"] * 16
            for (pcnt_, base, coff_, soff_) in mt["calls"]:
                qq = base // SEGW
                s0, s1 = bounds[qq], bounds[qq+1]
                loc = (sv[s0:s1] - base).astype(np.int16)
                sflat[base16 + soff_*16: base16 + soff_*16 + len(loc)] = loc
        sidxs.append(_pack16(sflat))
        uidxs.append(_pack16(uflat))
    return meta, sidxs, uidxs


def _build(meta=None, debug_no_gather=False, debug_no_collective=False, debug_ntiles=None):
    import concourse.bass as bass
    import concourse.tile as tile
    from concourse import bacc, mybir
    from concourse.masks import make_identity
    ntiles = NTILES if debug_ntiles is None else debug_ntiles

    f32 = mybir.dt.float32
    i32 = mybir.dt.int32

    nc = bacc.Bacc(None, target_bir_lowering=False, num_devices=NCORES,
                   dynamic_dma_scratch_size=65536)

    featsT = nc.dram_tensor("featsT", [C_IN, NT], f32, kind="ExternalInput")
    feats = nc.dram_tensor("feats", [NT, C_IN], f32, kind="ExternalInput")
    if meta is None:
        nbr = nc.dram_tensor("nbr", [NT, K], i32, kind="ExternalInput")
    else:
        i16 = mybir.dt.int16
        SIW = meta["siw"]          # total sidx cols
        UIW = NTILES * (SLOTS // 16)
        sidx = nc.dram_tensor("sidx", [P, SIW], i16, kind="ExternalInput")
        uidx = nc.dram_tensor("uidx", [P, UIW], i16, kind="ExternalInput")
    W1 = nc.dram_tensor("W1", [C_IN, C_MID], f32, kind="ExternalInput")
    W2f = nc.dram_tensor("W2f", [KC, C_MID], f32, kind="ExternalInput")
    W3 = nc.dram_tensor("W3", [C_MID, C_OUT], f32, kind="ExternalInput")
    out = nc.dram_tensor("out", [NT, C_OUT], f32, kind="ExternalOutput")

    with tile.TileContext(nc) as tc:
        with (
            tc.tile_pool(name="dram", bufs=1, space="DRAM") as dram,
            tc.tile_pool(name="consts", bufs=1) as consts,
            tc.tile_pool(name="io1", bufs=3) as io1,
            tc.tile_pool(name="ln", bufs=4) as lnp,
            tc.tile_pool(name="gp", bufs=2) as gp,
            tc.tile_pool(name="gt", bufs=3) as gtp,
            tc.tile_pool(name="io3", bufs=3) as io3,
            tc.tile_pool(name="ps1", bufs=2, space="PSUM") as ps1,
            tc.tile_pool(name="pst", bufs=2, space="PSUM") as pst,
            tc.tile_pool(name="ps2", bufs=2, space="PSUM") as ps2,
            tc.tile_pool(name="ps3", bufs=2, space="PSUM") as ps3,
        ):
            h_shard = dram.tile([NT, C_MID], f32)
            h_full = dram.tile([N, C_MID], f32)

            # constants
            W1s = consts.tile([P, 2, C_MID], f32)
            nc.sync.dma_start(out=W1s[:, 0, :], in_=W1[0:P, :])
            nc.sync.dma_start(out=W1s[:, 1, :], in_=W1[P:2*P, :])
            W2s = consts.tile([P, NCHUNK, C_MID], f32)
            for j in range(NCHUNK):
                w = min(P, KC - j * P)
                nc.sync.dma_start(out=W2s[:w, j, :], in_=W2f[j*P:j*P+w, :])
            W3s = consts.tile([C_MID, C_OUT], f32)
            nc.sync.dma_start(out=W3s[:, :], in_=W3[:, :])
            ident = consts.tile([P, P], f32)
            make_identity(nc, ident[:])
            epst = consts.tile([P, 1], f32)
            nc.vector.memset(epst[:], EPS)

            def layernorm(x_ap, o_ap, T, relu):
                """o = LN(x) over the free dim (gamma=1, beta=0), optional relu."""
                stats = lnp.tile([P, 6], f32, tag="stats")
                mv = lnp.tile([P, 2], f32, tag="mv")
                nc.vector.bn_stats(out=stats[:T, :], in_=x_ap)
                nc.vector.bn_aggr(out=mv[:T, :], in_=stats[:T, :])
                rstd = lnp.tile([P, 1], f32, tag="rstd")
                nc.scalar.activation(
                    out=rstd[:T, :], in_=mv[:T, 1:2],
                    func=mybir.ActivationFunctionType.Sqrt,
                    bias=epst[:T], scale=1.0, alpha=0.0)
                nc.vector.reciprocal(out=rstd[:T, :], in_=rstd[:T, :])
                nc.vector.tensor_scalar(
                    out=o_ap, in0=x_ap,
                    scalar1=mv[:T, 0:1], scalar2=rstd[:T, :],
                    op0=mybir.AluOpType.subtract, op1=mybir.AluOpType.mult)
                if relu:
                    nc.scalar.activation(
                        out=o_ap, in_=o_ap,
                        func=mybir.ActivationFunctionType.Relu)

            # ---------------- phase 1: conv1 ----------------
            for t in range(ntiles):
                r0 = t * P
                T = min(P, NT - r0)
                fT = io1.tile([P, 2, P], f32, tag="fT")
                nc.sync.dma_start(out=fT[:, 0, :T], in_=featsT[0:P, r0:r0+T])
                nc.sync.dma_start(out=fT[:, 1, :T], in_=featsT[P:2*P, r0:r0+T])
                psum1 = ps1.tile([P, C_MID], f32, tag="psum1")
                for j in range(2):
                    nc.tensor.matmul(
                        out=psum1[:T, :], lhsT=fT[:, j, :T], rhs=W1s[:, j, :],
                        start=(j == 0), stop=(j == 1))
                h_t = io1.tile([P, C_MID], f32, tag="h_t")
                layernorm(psum1[:T, :], h_t[:T, :], T, relu=True)
                nc.sync.dma_start(out=h_shard[r0:r0+T, :], in_=h_t[:T, :])

            # ---------------- phase 2: allgather ----------------
            if debug_no_collective:
                for c in range(NCORES):
                    nc.sync.dma_start(out=h_full[c*NT:(c+1)*NT, :][0:NT, :],
                                      in_=h_shard[:, :]) if c == 0 else None
            else:
                nc.gpsimd.collective_compute(
                    "AllGather", mybir.AluOpType.bypass,
                    replica_groups=[list(range(NCORES))],
                    ins=[h_shard[:, :].opt()],
                    outs=[h_full[:, :].opt()],
                )

            # ---------------- phase 3: conv2 + conv3 ----------------
            if meta is not None:
                gsem = nc.alloc_semaphore("gsem")
                _cnt = [0]
            for t in range(ntiles):
                r0 = t * P
                T = min(P, NT - r0)
                G = gp.tile([P, K, C_MID], f32, tag="G")
                if meta is not None:
                    tmeta = meta["tiles"][t]
                    scol0 = tmeta["scol0"]
                    tcols = tmeta["tcols"]
                    sid_t = io3.tile([P, meta["max_tcols"]], i16, tag="sid")
                    uid_t = io3.tile([P, SLOTS // 16], i16, tag="uid")
                    Gs = gp.tile([P, SCHUNK, C_MID], f32, tag="Gs")
                    scr = dram.tile([P * SCHUNK, C_MID], f32, tag="scr", bufs=2)
                    import os
                    _gm = int(os.environ.get("GATHER_MODE", "2"))
                    nc.sync.dma_start(out=sid_t[:, :tcols],
                                      in_=sidx[:, scol0:scol0 + tcols])
                    nc.sync.dma_start(
                        out=uid_t[:, :],
                        in_=uidx[:, t * (SLOTS // 16):(t + 1) * (SLOTS // 16)])
                    with tc.tile_critical():
                        _c = _cnt[0]
                        ncalls = 0
                        for (pcnt, base, coff, soff) in tmeta["calls"]:
                            if _gm >= 1:
                                nc.gpsimd.dma_gather(
                                    Gs[:, coff:coff + pcnt // P, :],
                                    h_full[base:N, :],
                                    sid_t[:, soff:soff + pcnt // 16],
                                    pcnt, pcnt, C_MID,
                                    single_packet=False,
                                ).then_inc(gsem, 16)
                            else:
                                nc.gpsimd.dma_start(
                                    out=Gs[:, coff, :],
                                    in_=h_full[0:P, 0:C_MID],
                                ).then_inc(gsem, 16)
                            ncalls += 1
                        _c += 16 * ncalls
                        nc.gpsimd.wait_ge(gsem, _c)
                        nc.gpsimd.dma_start(
                            out=scr[:, :].rearrange("(p c) d -> p c d", p=P),
                            in_=Gs[:, :, :]).then_inc(gsem, 16)
                        _c += 16
                        nc.gpsimd.wait_ge(gsem, _c)
                        if _gm >= 2:
                            nc.gpsimd.dma_gather(
                                G[:, :, :],
                                scr[:, :],
                                uid_t[:, :],
                                SLOTS, SLOTS, C_MID,
                                single_packet=False,
                            ).then_inc(gsem, 16)
                        else:
                            nc.gpsimd.dma_start(
                                out=G[:, :, :].rearrange("p k d -> p (k d)"),
                                in_=scr[:, :].rearrange(
                                    "(p c) d -> p c d", p=P)[:, 0:K, :]
                                    .rearrange("p c d -> p (c d)"),
                            ).then_inc(gsem, 16)
                        _c += 16
                        nc.gpsimd.wait_ge(gsem, _c)
                        _cnt[0] = _c
                elif debug_no_gather:
                    idx_t = io3.tile([P, K], i32, tag="idx")
                    nc.sync.dma_start(out=idx_t[:T, :], in_=nbr[r0:r0+T, :])
                    nc.sync.dma_start(
                        out=G[:T].rearrange("p k d -> p (k d)")[:, 0:C_MID],
                        in_=h_full[r0:r0+T, :])
                    nc.vector.memset(G[:T, 1:K, :], 0.01)
                else:
                    idx_t = io3.tile([P, K], i32, tag="idx")
                    nc.sync.dma_start(out=idx_t[:T, :], in_=nbr[r0:r0+T, :])
                    for k in range(K):
                        nc.gpsimd.indirect_dma_start(
                            out=G[:T, k, :], out_offset=None,
                            in_=h_full[:, :],
                            in_offset=bass.IndirectOffsetOnAxis(
                                ap=idx_t[:T, k:k+1], axis=0))
                Gf = G[:T].rearrange("p k d -> p (k d)")
                psum2 = ps2.tile([P, C_MID], f32, tag="psum2")
                for j in range(NCHUNK):
                    w = min(P, KC - j * P)
                    ps_t = pst.tile([P, P], f32, tag="ps_t")
                    nc.tensor.transpose(
                        out=ps_t[:w, :T], in_=Gf[:, j*P:j*P+w],
                        identity=ident[:T, :T])
                    gt = gtp.tile([P, P], f32, tag="gt")
                    nc.vector.tensor_copy(out=gt[:w, :T], in_=ps_t[:w, :T])
                    nc.tensor.matmul(
                        out=psum2[:T, :], lhsT=gt[:w, :T], rhs=W2s[:w, j, :],
                        start=(j == 0), stop=(j == NCHUNK - 1))
                h2 = io3.tile([P, C_MID], f32, tag="h2")
                layernorm(psum2[:T, :], h2[:T, :], T, relu=True)
                ps_h2t = pst.tile([P, P], f32, tag="ps_t")
                nc.tensor.transpose(
                    out=ps_h2t[:C_MID, :T], in_=h2[:T, :],
                    identity=ident[:T, :T])
                h2t = io3.tile([C_MID, P], f32, tag="h2t")
                nc.vector.tensor_copy(out=h2t[:, :T], in_=ps_h2t[:C_MID, :T])
                psum3 = ps3.tile([P, C_OUT], f32, tag="psum3")
                nc.tensor.matmul(
                    out=psum3[:T, :], lhsT=h2t[:, :T], rhs=W3s[:, :],
                    start=True, stop=True)
                o_t = io3.tile([P, C_OUT], f32, tag="o_t")
                layernorm(psum3[:T, :], o_t[:T, :], T, relu=False)
                f_t = io3.tile([P, C_IN], f32, tag="f_t")
                nc.sync.dma_start(out=f_t[:T, :], in_=feats[r0:r0+T, :])
                nc.vector.tensor_add(out=o_t[:T, :], in0=o_t[:T, :], in1=f_t[:T, :])
                nc.scalar.activation(
                    out=o_t[:T, :], in_=o_t[:T, :],
                    func=mybir.ActivationFunctionType.Relu)
                nc.sync.dma_start(out=out[r0:r0+T, :], in_=o_t[:T, :])

    nc.compile()
    return nc


def _make_runner(nc, n_cores):
    import jax
    from jax.sharding import Mesh, PartitionSpec
    from jax.experimental.shard_map import shard_map
    import concourse.mybir as mybir
    from concourse.bass2jax import (
        _bass_exec_p, install_neuronx_cc_hook, partition_id_tensor)

    install_neuronx_cc_hook()
    partition_name = nc.partition_id_tensor.name if nc.partition_id_tensor else None

    in_names, out_names, out_avals, zero_outs = [], [], [], []
    for alloc in nc.m.functions[0].allocations:
        if not isinstance(alloc, mybir.MemoryLocationSet):
            continue
        name = alloc.memorylocations[0].name
        if alloc.kind == "ExternalInput":
            if name != partition_name:
                in_names.append(name)
        elif alloc.kind == "ExternalOutput":
            shape = tuple(alloc.tensor_shape)
            dtype = mybir.dt.np(alloc.dtype)
            out_avals.append(jax.core.ShapedArray(shape, dtype))
            out_names.append(name)
            zero_outs.append(np.zeros(shape, dtype))
    n_params = len(in_names)
    n_outs = len(out_avals)
    all_in_names = list(in_names) + list(out_names)
    if partition_name is not None:
        all_in_names.append(partition_name)
    donate = tuple(range(n_params, n_params + n_outs))

    def _body(*args):
        operands = list(args)
        if partition_name is not None:
            operands.append(partition_id_tensor())
        outs = _bass_exec_p.bind(
            *operands,
            out_avals=tuple(out_avals),
            in_names=tuple(all_in_names),
            out_names=tuple(out_names),
            lowering_input_output_aliases=(),
            sim_require_finite=True,
            sim_require_nnan=True,
            nc=nc,
        )
        return tuple(outs)

    devices = jax.devices()[:n_cores]
    mesh = Mesh(np.asarray(devices), ("core",))
    in_specs = (PartitionSpec("core"),) * (n_params + n_outs)
    out_specs = (PartitionSpec("core"),) * n_outs
    sharded = jax.jit(
        shard_map(_body, mesh=mesh, in_specs=in_specs, out_specs=out_specs,
                  check_rep=False),
        donate_argnums=donate, keep_unused=True,
    )

    def fn(in_maps):
        per_core = [[np.asarray(m[name]) for name in in_names] for m in in_maps]
        concat_in = [np.concatenate([per_core[c][i] for c in range(n_cores)], axis=0)
                     for i in range(n_params)]
        concat_zeros = [np.zeros((n_cores * z.shape[0], *z.shape[1:]), z.dtype)
                        for z in zero_outs]
        out_arrs = sharded(*concat_in, *concat_zeros)
        out_arrs = [np.asarray(a) for a in out_arrs]
        return [
            {name: out_arrs[i].reshape(n_cores, *out_avals[i].shape)[c]
             for i, name in enumerate(out_names)}
            for c in range(n_cores)
        ]

    return fn


def _get_runner():
    if "fn" not in _RUNNER:
        nc = _build()
        _RUNNER["fn"] = _make_runner(nc, NCORES)
    return _RUNNER["fn"]


def kernel(feats, neighbor_idx, W1, g1, b1, W2, g2, b2, W3, g3, b3):
    feats = np.asarray(feats, dtype=np.float32)
    neighbor_idx = np.asarray(neighbor_idx, dtype=np.int32)
    W1 = np.asarray(W1, dtype=np.float32)
    W2 = np.asarray(W2, dtype=np.float32)
    W3 = np.asarray(W3, dtype=np.float32)
    W2f = np.ascontiguousarray(W2.reshape(KC, C_MID))
    featsT = np.ascontiguousarray(feats.T)

    import os
    fast = os.environ.get("FAST_GATHER", "0") == "1"
    if fast:
        meta, sidxs, uidxs = _prep_gather(neighbor_idx)
        sig = (meta["siw"],
               tuple(tuple(mt["calls"]) for mt in meta["tiles"]))
    else:
        meta, sig = None, "indirect"
    if _RUNNER.get("sig") != sig:
        nc = _build(meta=meta)
        _RUNNER["fn"] = _make_runner(nc, NCORES)
        _RUNNER["sig"] = sig
    fn = _RUNNER["fn"]

    in_maps = []
    for c in range(NCORES):
        sl = slice(c * NT, (c + 1) * NT)
        m = {
            "featsT": np.ascontiguousarray(featsT[:, sl]),
            "feats": feats[sl],
            "W1": W1, "W2f": W2f, "W3": W3,
        }
        if fast:
            m["sidx"], m["uidx"] = sidxs[c], uidxs[c]
        else:
            m["nbr"] = neighbor_idx[sl]
        in_maps.append(m)
    res = fn(in_maps)
    return np.concatenate([res[c]["out"] for c in range(NCORES)], axis=0)
